# revision 1
# baseline (speedup 1.0000x reference)
"""Trainium2 Bass kernel for nn_BaseKernelSetConv (gnn_message_passing).

Strategy (8 NeuronCores, data-parallel over nodes):
  - Focal scores: computed DENSELY. Node i appears as a focal exactly once
    (the per-degree selected_index sets partition all nodes), so each core
    streams its contiguous 125k-node shard of x, normalizes rows on-chip,
    transposes 128-node blocks on the PE, and matmuls against all four
    focal kernel sets at once -> (64, shard) scores. The host later keeps
    only the 16-wide band matching each node's actual degree.
  - Neighbor scores: per (core, degree) the neighbor rows are gathered with
    [128,1]-form indirect DMAs (one row per partition per instruction - the
    only indirect form the SWDGE ucode implements correctly), normalized,
    transposed per 128-node block and matmuled against the stacked
    (unit-normalized, /deg) neighbor kernels -> (16, n_d) scores.
  - Host assembles: res[node, band(deg)] = focal_band + neighbor_scores.
"""

import sys
import numpy as np

sys.path.insert(0, "/opt/trn_rl_repo")

F = 32
K = 16
NCORES = 8

_PROG = None


def _chunks(total_nodes, g):
    """Split total_nodes (multiple of 128) into (start, G) chunks of
    128*G nodes with a possibly smaller tail."""
    out = []
    start = 0
    nb = total_nodes // 128
    while start < nb:
        gc = min(g, nb - start)
        out.append((start * 128, gc))
        start += gc
    return out


def _configure(n, lshard, npad, gf, ndc, gsup):
    global N, SHARD, LSHARD, NPAD, GF, NDC, GSUP, FOCAL_CHUNKS, NEI_CHUNKS, _PROG
    N = n
    SHARD = n // NCORES
    LSHARD = lshard                   # dense shard padded to mult of 128
    NPAD = npad                       # padded table rows
    GF = gf                           # focal: nodes per partition per chunk
    NDC = dict(ndc)                   # padded nodes per (core, degree)
    GSUP = dict(gsup)                 # neighbor: nodes/partition/supertile
    FOCAL_CHUNKS = _chunks(LSHARD, GF)
    NEI_CHUNKS = {d: _chunks(NDC[d], GSUP[d]) for d in (1, 2, 3, 4)}
    _PROG = None


_configure(1_000_000, 125056, 1000576, 32,
           {1: 25728, 2: 38016, 3: 38016, 4: 25728},
           {1: 48, 2: 24, 3: 16, 4: 12})


def _set_ndc(ndc):
    """Tighten padded per-(core,degree) node counts to the actual input
    (one gather instruction per 128 wasted pad rows otherwise)."""
    global NDC, NEI_CHUNKS, _PROG
    if dict(ndc) != NDC:
        NDC = dict(ndc)
        NEI_CHUNKS = {d: _chunks(NDC[d], GSUP[d]) for d in (1, 2, 3, 4)}
        _PROG = None


def _build_program():
    import concourse.bass as bass
    import concourse.tile as tile
    from concourse import bacc, mybir
    from concourse.masks import make_identity

    f32 = mybir.dt.float32
    i32 = mybir.dt.int32
    AX = mybir.AxisListType.X
    MUL = mybir.AluOpType.mult
    ADD = mybir.AluOpType.add

    nc = bacc.Bacc("TRN2", target_bir_lowering=False, debug=False,
                   num_devices=NCORES)
    x_d = nc.dram_tensor("x", (NPAD, F), f32, kind="ExternalInput").ap()
    xsh_d = nc.dram_tensor("xsh", (LSHARD, F), f32, kind="ExternalInput").ap()
    wf_d = nc.dram_tensor("wf", (F, 64), f32, kind="ExternalInput").ap()
    wn_d = {d: nc.dram_tensor(f"wn{d}", (d * F, K), f32,
                              kind="ExternalInput").ap() for d in (1, 2, 3, 4)}
    idx_d = {d: nc.dram_tensor(f"idx{d}", (NDC[d] * d,), i32,
                               kind="ExternalInput").ap() for d in (1, 2, 3, 4)}
    focal_o = nc.dram_tensor("focal_o", (64, LSHARD), f32,
                             kind="ExternalOutput").ap()
    nei_o = {d: nc.dram_tensor(f"nei_o{d}", (K, NDC[d]), f32,
                               kind="ExternalOutput").ap() for d in (1, 2, 3, 4)}

    with tile.TileContext(nc) as tc:
        with tc.tile_pool(name="wp", bufs=1) as wp, \
             tc.tile_pool(name="stage", bufs=3) as stage_p, \
             tc.tile_pool(name="scr", bufs=2) as scr_p, \
             tc.tile_pool(name="nrm", bufs=2) as nrm_p, \
             tc.tile_pool(name="idxp", bufs=3) as idx_p, \
             tc.tile_pool(name="tsb", bufs=3) as tsb_p, \
             tc.tile_pool(name="ost", bufs=2) as ost_p, \
             tc.tile_pool(name="fstage", bufs=2) as fstage_p, \
             tc.tile_pool(name="fscr", bufs=2) as fscr_p, \
             tc.tile_pool(name="fnrm", bufs=2) as fnrm_p, \
             tc.tile_pool(name="ftsb", bufs=3) as ftsb_p, \
             tc.tile_pool(name="fost", bufs=2) as fost_p, \
             tc.tile_pool(name="tps", bufs=2, space="PSUM") as tps_p, \
             tc.tile_pool(name="ftps", bufs=2, space="PSUM") as ftps_p, \
             tc.tile_pool(name="fps", bufs=2, space="PSUM") as fps_p, \
             tc.tile_pool(name="sps", bufs=2, space="PSUM") as sps_p:

            ident = wp.tile([128, 128], f32)
            make_identity(nc, ident[:])
            wf_sb = wp.tile([F, 64], f32, tag="wf")
            nc.sync.dma_start(wf_sb[:], wf_d[:])
            wn_sb = {}
            for d in (1, 2, 3, 4):
                wn_sb[d] = wp.tile([d * F, K], f32, tag=f"wn{d}",
                                   name=f"wn_sb{d}")
                nc.sync.dma_start(wn_sb[d][:], wn_d[d][:])

            def normalize(st, rows, scrp, nrmp, pfx):
                """st: [128, rows*F] raw rows -> returns [128, rows*F]
                normalized tile (unit L2 per 32-float row)."""
                scr = scrp.tile([128, rows * F], f32, tag=pfx + "scr",
                                name=pfx + "scr")
                nc.scalar.square(scr[:], st[:])
                n2 = nrmp.tile([128, rows], f32, tag=pfx + "n2",
                               name=pfx + "n2")
                nc.vector.tensor_reduce(
                    n2[:], scr[:].rearrange("p (r f) -> p r f", f=F),
                    axis=AX, op=ADD)
                r2 = nrmp.tile([128, rows], f32, tag=pfx + "r2",
                               name=pfx + "r2")
                nc.vector.reciprocal(r2[:], n2[:])
                inv = nrmp.tile([128, rows], f32, tag=pfx + "inv",
                                name=pfx + "inv")
                nc.scalar.sqrt(inv[:], r2[:])
                nc.vector.tensor_tensor(
                    out=scr[:].rearrange("p (r f) -> p r f", f=F),
                    in0=st[:].rearrange("p (r f) -> p r f", f=F),
                    in1=inv[:].rearrange("p (r u) -> p r u", u=1)
                        .to_broadcast([128, rows, F]),
                    op=MUL)
                return scr

            focal_cols = {}
            c = 0
            for start, gc in FOCAL_CHUNKS:
                focal_cols[start] = c
                c += gc * 128
            nei_cols = {}
            for d in (1, 2, 3, 4):
                c = 0
                for start, gs in NEI_CHUNKS[d]:
                    nei_cols[(d, start)] = c
                    c += gs * 128

            def emit_focal(start, gc):
                col = focal_cols[start]
                st = fstage_p.tile([128, gc * F], f32, tag="fstage",
                                   name="fstage")
                nc.sync.dma_start(
                    st[:],
                    xsh_d[start:start + 128 * gc, :]
                        .rearrange("(p g) f -> p (g f)", p=128))
                nrmed = normalize(st, gc, fscr_p, fnrm_p, "f")
                ost = fost_p.tile([64, gc * 128], f32, tag="fost", name="fost")
                for g in range(gc):
                    tp = ftps_p.tile([128, 128], f32, tag="ftps", name="ftps")
                    nc.tensor.transpose(
                        out=tp[:F, :], in_=nrmed[:, g * F:(g + 1) * F],
                        identity=ident[:])
                    ts = ftsb_p.tile([128, 128], f32, tag="ftsb", name="ftsb")
                    nc.any.tensor_copy(ts[:F, :], tp[:F, :])
                    fp = fps_p.tile([64, 128], f32, tag="fps", name="fps")
                    nc.tensor.matmul(fp[:], lhsT=wf_sb[:], rhs=ts[:F, :],
                                     start=True, stop=True)
                    nc.any.tensor_copy(ost[:, g * 128:(g + 1) * 128], fp[:])
                nc.sync.dma_start(focal_o[:, col:col + gc * 128], ost[:])

            def emit_nei(d, start, gs):
                col = nei_cols[(d, start)]
                rows = gs * d
                it = idx_p.tile([128, rows], i32, tag="idx", name="idx")
                nc.sync.dma_start(
                    it[:],
                    idx_d[d][start * d:(start + 128 * gs) * d]
                        .rearrange("(p r) -> p r", p=128))
                st = stage_p.tile([128, rows * F], f32, tag="stage",
                                  name="stage")
                for r in range(rows):
                    nc.gpsimd.indirect_dma_start(
                        out=st[:, r * F:(r + 1) * F],
                        out_offset=None,
                        in_=x_d[:],
                        in_offset=bass.IndirectOffsetOnAxis(
                            ap=it[:, r:r + 1], axis=0),
                    )
                nrmed = normalize(st, rows, scr_p, nrm_p, "n")
                ost = ost_p.tile([K, gs * 128], f32, tag="ost", name="ost")
                for g in range(gs):
                    tp = tps_p.tile([128, 128], f32, tag="tps", name="tps")
                    nc.tensor.transpose(
                        out=tp[:d * F, :],
                        in_=nrmed[:, g * d * F:(g + 1) * d * F],
                        identity=ident[:])
                    ts = tsb_p.tile([128, 128], f32, tag="tsb", name="tsb")
                    nc.any.tensor_copy(ts[:d * F, :], tp[:d * F, :])
                    sp = sps_p.tile([K, 128], f32, tag="sps", name="sps")
                    nc.tensor.matmul(sp[:], lhsT=wn_sb[d][:],
                                     rhs=ts[:d * F, :],
                                     start=True, stop=True)
                    nc.any.tensor_copy(ost[:, g * 128:(g + 1) * 128], sp[:])
                nc.sync.dma_start(nei_o[d][:, col:col + gs * 128], ost[:])

            # Interleave: neighbor supertiles carry the critical path (Pool
            # descriptor generation); focal chunks slot into idle engines.
            # Small (tail) supertiles are emitted last so the post-final-
            # gather drain is as short as possible; focal chunks are front-
            # loaded to finish well inside the gather shadow.
            nei_items = [(d, s, g) for d in (1, 2, 3, 4)
                         for s, g in NEI_CHUNKS[d]]
            nei_items.sort(key=lambda t: -(t[2] * t[0]))   # big gathers first
            focal_items = list(FOCAL_CHUNKS)
            fi = 0
            pace = max(1, (len(nei_items) * 3) // 4)       # done by ~75% mark
            for i, (d, s, g) in enumerate(nei_items):
                emit_nei(d, s, g)
                while fi < len(focal_items) and fi + 1 <= (i + 1) * len(focal_items) // pace:
                    emit_focal(*focal_items[fi])
                    fi += 1
            while fi < len(focal_items):
                emit_focal(*focal_items[fi])
                fi += 1

    nc.compile()
    return nc


def _unit_rows(a):
    a = a.astype(np.float64)
    return (a / (np.linalg.norm(a, axis=-1, keepdims=True) + 1e-8)).astype(np.float32)


def host_prep(inputs):
    """Build per-core device inputs + bookkeeping for assembly."""
    x = np.ascontiguousarray(np.asarray(inputs["x"], dtype=np.float32))
    sels = {d: np.asarray(inputs[f"selected_index_deg{d}"]).astype(np.int64)
            for d in (1, 2, 3, 4)}
    neis = {d: np.asarray(inputs[f"nei_index_deg{d}"]).astype(np.int64)
            .reshape(-1, d) for d in (1, 2, 3, 4)}

    xpad = np.ones((NPAD, F), np.float32)   # pad rows finite (avoid 0-norm NaN)
    xpad[:N] = x

    deg = np.zeros(N, np.int8)
    pos = np.zeros(N, np.int64)
    for d in (1, 2, 3, 4):
        deg[sels[d]] = d
        pos[sels[d]] = np.arange(sels[d].shape[0])

    # weights
    wf_all = np.concatenate(
        [_unit_rows(np.asarray(inputs[f"W_focal{d}"], np.float32))
         for d in (1, 2, 3, 4)], axis=0)            # (64, 32)
    wf_lhsT = np.ascontiguousarray(wf_all.T)        # (32, 64)
    wn_lhsT = {}
    for d in (1, 2, 3, 4):
        wn = np.asarray(inputs[f"W_nei{d}"], np.float32)   # (16, d, 32)
        u = _unit_rows(wn.reshape(-1, F)).reshape(K, d, F) / d
        wn_lhsT[d] = np.ascontiguousarray(u.reshape(K, d * F).T)  # (d*32, 16)

    # tighten per-(core,degree) padding to the actual degree distribution
    all_nodes = {}
    maxcnt = {d: 0 for d in (1, 2, 3, 4)}
    for c in range(NCORES):
        lo, hi = c * SHARD, (c + 1) * SHARD
        shard_deg = deg[lo:hi]
        for d in (1, 2, 3, 4):
            nodes_cd = np.nonzero(shard_deg == d)[0] + lo   # ascending ids
            all_nodes[(c, d)] = nodes_cd
            maxcnt[d] = max(maxcnt[d], nodes_cd.shape[0])
    _set_ndc({d: ((maxcnt[d] + 127) // 128) * 128 for d in (1, 2, 3, 4)})

    in_maps = []
    book = []      # per core: {d: (nodes_cd, cnt)}
    for c in range(NCORES):
        lo, hi = c * SHARD, (c + 1) * SHARD
        xsh = xpad[lo:lo + LSHARD]
        m = {"x": xpad, "xsh": np.ascontiguousarray(xsh),
             "wf": wf_lhsT}
        bk = {}
        for d in (1, 2, 3, 4):
            m[f"wn{d}"] = wn_lhsT[d]
            nodes_cd = all_nodes[(c, d)]
            cnt = nodes_cd.shape[0]
            assert cnt <= NDC[d], (c, d, cnt)
            nei_cd = np.zeros((NDC[d], d), np.int32)
            nei_cd[:cnt] = neis[d][pos[nodes_cd]].astype(np.int32)
            # device layout: per supertile s (start,gs): [128, gs*d] with
            # (p, g*d+slot) = nei of node_local start + p*gs + g
            flat = np.empty(NDC[d] * d, np.int32)
            o = 0
            for start, gs in NEI_CHUNKS[d]:
                slab = nei_cd[start:start + 128 * gs].reshape(128, gs * d)
                flat[o:o + slab.size] = slab.reshape(-1)
                o += slab.size
            m[f"idx{d}"] = flat
            bk[d] = (nodes_cd, cnt)
        in_maps.append(m)
        book.append(bk)
    return in_maps, book


def _uninterleave(arr, chunks):
    """arr: (B, total_cols) device-order -> (total_nodes, B) node-local order.
    Device col order per chunk: g*128 + p ; node_local = start + p*gc + g."""
    b = arr.shape[0]
    total = sum(gc * 128 for _, gc in chunks)
    out = np.empty((total, b), arr.dtype)
    col = 0
    for start, gc in chunks:
        blk = arr[:, col:col + gc * 128].reshape(b, gc, 128)
        out[start:start + gc * 128] = blk.transpose(2, 1, 0).reshape(gc * 128, b)
        col += gc * 128
    return out


def assemble(results, book):
    res = np.zeros((N, 64), np.float32)
    for c in range(NCORES):
        lo = c * SHARD
        focal = _uninterleave(results[c]["focal_o"], FOCAL_CHUNKS)  # (LSHARD, 64)
        for d in (1, 2, 3, 4):
            nodes_cd, cnt = book[c][d]
            nei = _uninterleave(results[c][f"nei_o{d}"], NEI_CHUNKS[d])  # (NDC,16)
            band = slice(16 * (d - 1), 16 * d)
            res[nodes_cd, band] = focal[nodes_cd - lo, band] + nei[:cnt]
    return res


LAST_RESULTS = None


def kernel(**inputs):
    global _PROG, LAST_RESULTS
    import os
    from concourse.bass_utils import run_bass_kernel_spmd
    in_maps, book = host_prep(inputs)   # may retune NDC -> resets _PROG
    if _PROG is None:
        _PROG = _build_program()
    trace = bool(os.environ.get("BKC_TRACE"))
    res = run_bass_kernel_spmd(_PROG, in_maps, core_ids=list(range(NCORES)),
                               trace=trace)
    LAST_RESULTS = res
    return assemble(res.results, book)


# ---------------------------------------------------------------------------
# numpy emulation of the device program (for fast host-logic validation)
def _emulate_core(m):
    x = m["x"]
    out = {}
    xs = m["xsh"].astype(np.float64)
    y = xs / (np.sqrt((xs * xs).sum(-1, keepdims=True)))
    z = (y @ m["wf"].astype(np.float64))            # (LSHARD, 64)
    focal = np.empty((64, LSHARD), np.float32)
    col = 0
    for start, gc in FOCAL_CHUNKS:
        blk = z[start:start + 128 * gc].reshape(128, gc, 64)
        focal[:, col:col + gc * 128] = (
            blk.transpose(2, 1, 0).reshape(64, gc * 128))
        col += gc * 128
    out["focal_o"] = focal
    for d in (1, 2, 3, 4):
        flat = m[f"idx{d}"]
        nei_out = np.empty((K, NDC[d]), np.float32)
        col = 0
        o = 0
        for start, gs in NEI_CHUNKS[d]:
            slab = flat[o:o + 128 * gs * d].reshape(128, gs, d)
            o += 128 * gs * d
            g = x[slab].astype(np.float64)          # (128, gs, d, 32)
            g = g / np.sqrt((g * g).sum(-1, keepdims=True))
            sc = np.einsum("pgdf,dfk->kgp", g,
                           m[f"wn{d}"].astype(np.float64).reshape(d, F, K))
            nei_out[:, col:col + gs * 128] = sc.reshape(K, gs * 128)
            col += gs * 128
        out[f"nei_o{d}"] = nei_out
    return out


def kernel_emulated(**inputs):
    in_maps, book = host_prep(inputs)
    results = [_emulate_core(m) for m in in_maps]
    return assemble(results, book)

